# revision 15
# baseline (speedup 1.0000x reference)
"""Trainium2 Bass kernel for BlockAttnRes.compute_all_inputs (bf16 pipeline).

v3: lean softmax path (no PE transposes, no mask-add), 1/ssum folded into
H copies, copies spread over scalar/vector/gpsimd, software-pipelined
emission. Input DMA on SP, output on scalar HWDGE.
"""

import numpy as np
import ml_dtypes

import concourse.bass as bass
import concourse.bacc as bacc
import concourse.mybir as mybir
from concourse import tile
from concourse.alu_op_type import AluOpType
from concourse.bass_utils import run_bass_kernel_spmd

L = 24
D = 2048
NUM_BLOCKS = 8
EPS = 1e-6
B, T = 2, 1024
N_CORES = 8

ROWS_PER_CORE = (B * T) // N_CORES  # 256
R = 5              # rows per batch
NJ = 25            # raw vectors per row: emb + 24 layer outputs
NS = 25            # sources per row
P = NJ * R         # 125 partitions per batch
RL = R * L         # 120
NCHUNK = D // 128  # 16 d-chunks
CW = 152           # vt per-chunk pitch: 128 (VT, cols 125:128 zero) + 24 qwT

f32 = mybir.dt.float32
bf16 = mybir.dt.bfloat16
BF = ml_dtypes.bfloat16


def _source_matrix():
    M = np.zeros((NS, NJ), dtype=np.float32)
    M[0, 0] = 1.0
    for k in range(NUM_BLOCKS):
        for i in range(3):
            M[1 + 3 * k + i, 1 + 3 * k : 1 + 3 * k + i + 1] = 1.0
    return M


def _valid_matrix():
    V = np.zeros((L, NS), dtype=bool)
    for l in range(L):
        kb, ii = l // 3, l % 3
        V[l, 0] = True
        for k in range(kb):
            V[l, 3 * k + 3] = True
        if ii > 0:
            V[l, 3 * kb + ii] = True
    return V


def _build_consts(queries, key_norm_weight):
    M = _source_matrix()
    valid = _valid_matrix()
    eye_r = np.eye(R, dtype=np.float32)

    qw = (queries * key_norm_weight[None, :]).astype(np.float32)  # [L, D]
    qwT = np.ascontiguousarray(
        qw.reshape(L, NCHUNK, 128).transpose(2, 1, 0).reshape(128, NCHUNK * L)
    ).astype(BF)

    # mtbd[(a,j), (b,n)] = M[n,j] * (a==b);  rows a*NJ+j, cols b*NS+n
    mtbd = np.einsum("nj,ab->ajbn", M, eye_r).reshape(P, NS * R)
    mtbd128 = np.zeros((P, 128), np.float32)
    mtbd128[:, :P] = mtbd
    mtbd128 = mtbd128.astype(BF)
    # mbd[(a,n), (b,j)] = M[n,j] * (a==b);  rows a*NS+n, cols b*NJ+j
    mbd = np.einsum("nj,ab->anbj", M, eye_r).reshape(NS * R, P)
    mbd128 = np.zeros((P, 128), np.float32)
    mbd128[:, :P] = mbd
    mbd128 = mbd128.astype(BF)
    # eyebd for diag extraction of the source gram
    eye_bd = np.zeros((P, 128), np.float32)
    eye_bd[:, :P] = np.eye(P, dtype=np.float32)
    # diagm2[(b,n), (r,l)] = (b==r) * valid[l, n]
    diagm2 = np.zeros((P, RL), np.float32)
    for b in range(R):
        for n in range(NS):
            for l in range(L):
                if valid[l, n]:
                    diagm2[b * NS + n, b * L + l] = 1.0
    diagm2 = diagm2.astype(BF)
    ones125 = np.ones((P, 1), np.float32).astype(BF)
    return dict(qwT=qwT, mtbd=mtbd128, mbd=mbd128, eyebd=eye_bd,
                diagm2=diagm2, ones125=ones125)


def _batch_starts():
    starts = [R * b for b in range(ROWS_PER_CORE // R)]  # 0..250
    if starts[-1] + R < ROWS_PER_CORE:
        starts.append(ROWS_PER_CORE - R)  # 251 (overlaps; identical rewrites)
    return starts


def build_kernel(do_compile=True):
    nc = bacc.Bacc("TRN2", target_bir_lowering=False, debug=False)

    loT = nc.dram_tensor("loT", [ROWS_PER_CORE * NJ, D], bf16,
                         kind="ExternalInput").ap()
    qwT_d = nc.dram_tensor("qwT", [128, NCHUNK * L], bf16, kind="ExternalInput").ap()
    mtbd_d = nc.dram_tensor("mtbd", [P, 128], bf16, kind="ExternalInput").ap()
    mbd_d = nc.dram_tensor("mbd", [P, 128], bf16, kind="ExternalInput").ap()
    eyebd_d = nc.dram_tensor("eyebd", [P, 128], f32, kind="ExternalInput").ap()
    diagm2_d = nc.dram_tensor("diagm2", [P, RL], bf16, kind="ExternalInput").ap()
    ones_d = nc.dram_tensor("ones125", [P, 1], bf16, kind="ExternalInput").ap()
    outT = nc.dram_tensor("outT", [ROWS_PER_CORE * L, D], bf16,
                          kind="ExternalOutput").ap()

    with tile.TileContext(nc) as tc:
        with (
            tc.tile_pool(name="const", bufs=1) as const,
            tc.tile_pool(name="xpool", bufs=6) as xpool,
            tc.tile_pool(name="hpool", bufs=3) as hpool,
            tc.tile_pool(name="spool", bufs=2) as spool,
            tc.tile_pool(name="ps_ft", bufs=3, space=bass.MemorySpace.PSUM) as ps_ft,
            tc.tile_pool(name="ps_sc", bufs=2, space=bass.MemorySpace.PSUM) as ps_sc,
            tc.tile_pool(name="ps_h", bufs=3, space=bass.MemorySpace.PSUM) as ps_h,
        ):
            mtbd = const.tile([P, 128], bf16)
            nc.sync.dma_start(mtbd[:], mtbd_d[:])
            mbd = const.tile([P, 128], bf16)
            nc.sync.dma_start(mbd[:], mbd_d[:])
            eyebd = const.tile([P, 128], f32)
            nc.sync.dma_start(eyebd[:], eyebd_d[:])
            diagm2 = const.tile([P, RL], bf16)
            nc.sync.dma_start(diagm2[:], diagm2_d[:])
            ones125 = const.tile([P, 1], bf16)
            nc.sync.dma_start(ones125[:], ones_d[:])
            epsb = const.tile([P, 1], f32)
            nc.vector.memset(epsb[:], EPS)
            junk = const.tile([P, P], f32)

            # two persistent vt tiles (ping-pong); qw block written once
            vt_a = const.tile([128, NCHUNK * CW], bf16)
            vt_b = const.tile([128, NCHUNK * CW], bf16)
            vts = [vt_a, vt_b]
            for v in vts:
                nc.sync.dma_start(
                    v.rearrange("p (c w) -> p c w", w=CW)[:, :, 128 : 128 + L],
                    qwT_d.rearrange("p (c w) -> p c w", w=L),
                )

            starts = _batch_starts()
            X_tiles = {}

            def load(j):
                if j >= len(starts):
                    return
                row0 = starts[j]
                X = xpool.tile([P, D], bf16, name=f"X{j}", tag="X")
                nc.gpsimd.dma_start(X[:], loT[row0 * NJ : row0 * NJ + P, :])
                X_tiles[j] = X

            def ft_part(i):
                X = X_tiles[i]
                vt3 = vts[i % 2].rearrange("p (c w) -> p c w", w=CW)
                for half in range(4):
                    ftp = ps_ft.tile([128, 512], f32, name=f"ftp{i}_{half}",
                                     tag="ft")
                    for cc in range(4):
                        c = 4 * half + cc
                        nc.tensor.matmul(
                            ftp[:, 128 * cc : 128 * (cc + 1)],
                            X[:, 128 * c : 128 * (c + 1)],
                            mtbd[:],
                            start=True,
                            stop=True,
                        )
                    ft4 = ftp.rearrange("p (cc w) -> p cc w", w=128)
                    dst = vt3[:, 4 * half : 4 * half + 4, 0:128]
                    if half % 2 == 0:
                        nc.scalar.copy(dst, ft4)
                    else:
                        nc.vector.tensor_copy(dst, ft4)

            def sc_part(i):
                vt3 = vts[i % 2].rearrange("p (c w) -> p c w", w=CW)
                # one PSUM bank per batch holds SC scores (0:152), BT
                # (160:280) and RS (280:281) in disjoint ranges
                combo = ps_sc.tile([128, 512], f32, name=f"combo{i}", tag="sc")
                SCp = combo[:, 0:CW]
                for c in range(NCHUNK):
                    nc.tensor.matmul(
                        SCp[0:P, :],
                        vt3[:, c, 0:P],
                        vt3[:, c, 0:CW],
                        start=(c == 0),
                        stop=(c == NCHUNK - 1),
                    )
                return combo

            def scores_part(i, combo):
                SCp = combo[:, 0:CW]
                ssq = spool.tile([P, 1], f32, name=f"ssq{i}", tag="ssq")
                nc.vector.scalar_tensor_tensor(
                    out=junk[:, 0:P],
                    in0=SCp[0:P, 0:P],
                    scalar=1.0,
                    in1=eyebd[:, 0:P],
                    op0=AluOpType.mult,
                    op1=AluOpType.mult,
                    accum_out=ssq[:],
                )
                lnu = spool.tile([P, 1], f32, name=f"lnu{i}", tag="lnu")
                nc.scalar.activation(
                    lnu[:], ssq[:], mybir.ActivationFunctionType.Ln,
                    bias=epsb[:], scale=1.0 / D,
                )
                rsq = spool.tile([P, 1], f32, name=f"rsq{i}", tag="rsq")
                nc.scalar.activation(
                    rsq[:], lnu[:], mybir.ActivationFunctionType.Exp, scale=-0.5
                )
                exp_s = spool.tile([P, L], bf16, name=f"exps{i}", tag="exps")
                nc.scalar.activation(
                    exp_s[:], SCp[0:P, 128 : 128 + L],
                    mybir.ActivationFunctionType.Exp, scale=rsq[:],
                )
                esc2 = spool.tile([P, RL], bf16, name=f"esc2{i}", tag="esc2")
                nc.gpsimd.tensor_tensor(
                    esc2.rearrange("p (r l) -> p r l", r=R),
                    exp_s.unsqueeze(1).broadcast_to([P, R, L]),
                    diagm2.rearrange("p (r l) -> p r l", r=R),
                    AluOpType.mult,
                )
                return esc2

            def bt_part(i, esc2, combo):
                bt = combo[:, 160:280]
                rs = combo[0:RL, 280:281]
                nc.tensor.matmul(bt[:], mbd[:], esc2[:],
                                 start=True, stop=True)
                nc.tensor.matmul(rs[:], esc2[:], ones125[:],
                                 start=True, stop=True)
                rec = spool.tile([RL, 1], f32, name=f"rec{i}", tag="rec")
                nc.vector.reciprocal(rec[:], rs[:])
                btsb = spool.tile([P, RL], bf16, name=f"btsb{i}", tag="btsb")
                nc.vector.tensor_copy(btsb[:], bt[0:P, :])
                return dict(rec=rec, btsb=btsb)

            def h_part(i, row0, bts):
                X, rec, btsb = X_tiles.pop(i), bts["rec"], bts["btsb"]
                h_sb = hpool.tile([RL, D], bf16, name=f"hsb{i}", tag="h")
                for nb in range(4):
                    Hp = ps_h.tile([128, 512], f32, name=f"Hp{i}_{nb}", tag="hp")
                    nc.tensor.matmul(
                        Hp[0:RL, :],
                        btsb[:],
                        X[:, 512 * nb : 512 * (nb + 1)],
                        start=True,
                        stop=True,
                    )
                    dst = h_sb[:, 512 * nb : 512 * (nb + 1)]
                    if nb < 2:
                        nc.scalar.activation(
                            dst, Hp[0:RL, :],
                            mybir.ActivationFunctionType.Copy, scale=rec[:],
                        )
                    else:
                        nc.vector.scalar_tensor_tensor(
                            out=dst,
                            in0=Hp[0:RL, :],
                            scalar=1.0,
                            in1=rec.broadcast_to([RL, 512]),
                            op0=AluOpType.mult,
                            op1=AluOpType.mult,
                        )
                nc.scalar.dma_start(
                    outT[row0 * L : row0 * L + RL, :], h_sb[:]
                )

            PF = 3  # X prefetch depth
            for j in range(PF):
                load(j)
            prev = None  # (i, esc2, combo)
            for i, row0 in enumerate(starts):
                ft_part(i)
                load(i + PF)
                if prev is not None:
                    bts = bt_part(prev[0], prev[1], prev[2])
                combo = sc_part(i)
                if prev is not None:
                    h_part(prev[0], starts[prev[0]], bts)
                esc2 = scores_part(i, combo)
                prev = (i, esc2, combo)
            bts = bt_part(prev[0], prev[1], prev[2])
            h_part(prev[0], starts[prev[0]], bts)

    real_gat = bacc.get_activation_tables
    AF = mybir.ActivationFunctionType

    def gat_pinned(arch):
        out = {}
        for name, fns in real_gat(arch).items():
            if name == "natural_log_exp_and_others":
                out[name] = set(fns)
            else:
                out[name] = {f for f in fns if f not in (AF.Ln, AF.Exp)}
        return out

    bacc.get_activation_tables = gat_pinned
    try:
        if do_compile:
            nc.compile()
    finally:
        bacc.get_activation_tables = real_gat
    return nc


_NC_CACHE = None


def _prep_loT(layer_outputs, embedding):
    loT = np.empty((B * T, NJ, D), dtype=BF)
    loT[:, 0, :] = embedding.reshape(B * T, D).astype(BF)
    loT[:, 1:, :] = (
        layer_outputs.reshape(L, B * T, D).transpose(1, 0, 2).astype(BF)
    )
    return loT


def _make_in_maps(layer_outputs, embedding, queries, key_norm_weight):
    loT = _prep_loT(layer_outputs, embedding)
    consts = _build_consts(queries, key_norm_weight)
    in_maps = []
    for c in range(N_CORES):
        r0 = c * ROWS_PER_CORE
        in_maps.append({
            "loT": loT[r0 : r0 + ROWS_PER_CORE].reshape(ROWS_PER_CORE * NJ, D),
            "qwT": consts["qwT"],
            "mtbd": consts["mtbd"],
            "mbd": consts["mbd"],
            "eyebd": consts["eyebd"],
            "diagm2": consts["diagm2"],
            "ones125": consts["ones125"],
        })
    return in_maps


def kernel(layer_outputs, embedding, queries, key_norm_weight):
    global _NC_CACHE
    layer_outputs = np.asarray(layer_outputs, dtype=np.float32)
    embedding = np.asarray(embedding, dtype=np.float32)
    queries = np.asarray(queries, dtype=np.float32)
    key_norm_weight = np.asarray(key_norm_weight, dtype=np.float32)

    in_maps = _make_in_maps(layer_outputs, embedding, queries, key_norm_weight)

    if _NC_CACHE is None:
        _NC_CACHE = build_kernel()
    nc = _NC_CACHE

    res = run_bass_kernel_spmd(nc, in_maps, core_ids=list(range(N_CORES)))

    full = np.empty((L, B * T, D), dtype=np.float32)
    for c in range(N_CORES):
        r0 = c * ROWS_PER_CORE
        outT = res.results[c]["outT"].astype(np.float32).reshape(
            ROWS_PER_CORE, L, D
        )
        full[:, r0 : r0 + ROWS_PER_CORE, :] = outT.transpose(1, 0, 2)
    return full.reshape(L, B, T, D)


# revision 20
# speedup vs baseline: 1.1936x; 1.1936x over previous
"""Trainium2 Bass kernel for BlockAttnRes.compute_all_inputs (bf16 pipeline).

v3: lean softmax path (no PE transposes, no mask-add), 1/ssum folded into
H copies, copies spread over scalar/vector/gpsimd, software-pipelined
emission. Input DMA on SP, output on scalar HWDGE.
"""

import numpy as np
import ml_dtypes

import concourse.bass as bass
import concourse.bacc as bacc
import concourse.mybir as mybir
from concourse import tile
from concourse.alu_op_type import AluOpType
from concourse.bass_utils import run_bass_kernel_spmd

L = 24
D = 2048
NUM_BLOCKS = 8
EPS = 1e-6
B, T = 2, 1024
N_CORES = 8

ROWS_PER_CORE = (B * T) // N_CORES  # 256
R = 5              # rows per batch
NJ = 25            # raw vectors per row: emb + 24 layer outputs
NS = 25            # sources per row
P = NJ * R         # 125 partitions per batch
RL = R * L         # 120
NCHUNK = D // 128  # 16 d-chunks
CW = 152           # vt per-chunk pitch: 128 (VT, cols 125:128 zero) + 24 qwT

f32 = mybir.dt.float32
bf16 = mybir.dt.bfloat16
BF = ml_dtypes.bfloat16


def _source_matrix():
    M = np.zeros((NS, NJ), dtype=np.float32)
    M[0, 0] = 1.0
    for k in range(NUM_BLOCKS):
        for i in range(3):
            M[1 + 3 * k + i, 1 + 3 * k : 1 + 3 * k + i + 1] = 1.0
    return M


def _valid_matrix():
    V = np.zeros((L, NS), dtype=bool)
    for l in range(L):
        kb, ii = l // 3, l % 3
        V[l, 0] = True
        for k in range(kb):
            V[l, 3 * k + 3] = True
        if ii > 0:
            V[l, 3 * kb + ii] = True
    return V


def _build_consts(queries, key_norm_weight):
    M = _source_matrix()
    valid = _valid_matrix()
    eye_r = np.eye(R, dtype=np.float32)

    qw = (queries * key_norm_weight[None, :]).astype(np.float32)  # [L, D]
    qwT = np.ascontiguousarray(
        qw.reshape(L, NCHUNK, 128).transpose(2, 1, 0).reshape(128, NCHUNK * L)
    ).astype(BF)

    # mtbd[(a,j), (b,n)] = M[n,j] * (a==b);  rows a*NJ+j, cols b*NS+n
    mtbd = np.einsum("nj,ab->ajbn", M, eye_r).reshape(P, NS * R)
    mtbd128 = np.zeros((P, 128), np.float32)
    mtbd128[:, :P] = mtbd
    mtbd128 = mtbd128.astype(BF)
    # mbd[(a,n), (b,j)] = M[n,j] * (a==b);  rows a*NS+n, cols b*NJ+j
    mbd = np.einsum("nj,ab->anbj", M, eye_r).reshape(NS * R, P)
    mbd128 = np.zeros((P, 128), np.float32)
    mbd128[:, :P] = mbd
    mbd128 = mbd128.astype(BF)
    # eyebd for diag extraction of the source gram
    eye_bd = np.zeros((P, 128), np.float32)
    eye_bd[:, :P] = np.eye(P, dtype=np.float32)
    # diagm2[(b,n), (r,l)] = (b==r) * valid[l, n]
    diagm2 = np.zeros((P, RL), np.float32)
    for b in range(R):
        for n in range(NS):
            for l in range(L):
                if valid[l, n]:
                    diagm2[b * NS + n, b * L + l] = 1.0
    diagm2 = diagm2.astype(BF)
    ones125 = np.ones((P, 1), np.float32).astype(BF)
    return dict(qwT=qwT, mtbd=mtbd128, mbd=mbd128, eyebd=eye_bd,
                diagm2=diagm2, ones125=ones125)


def _batch_starts():
    starts = [R * b for b in range(ROWS_PER_CORE // R)]  # 0..250
    if starts[-1] + R < ROWS_PER_CORE:
        starts.append(ROWS_PER_CORE - R)  # 251 (overlaps; identical rewrites)
    return starts


def build_kernel(do_compile=True):
    nc = bacc.Bacc("TRN2", target_bir_lowering=False, debug=False)

    loT = nc.dram_tensor("loT", [ROWS_PER_CORE * NJ, D], bf16,
                         kind="ExternalInput").ap()
    qwT_d = nc.dram_tensor("qwT", [128, NCHUNK * L], bf16, kind="ExternalInput").ap()
    mtbd_d = nc.dram_tensor("mtbd", [P, 128], bf16, kind="ExternalInput").ap()
    mbd_d = nc.dram_tensor("mbd", [P, 128], bf16, kind="ExternalInput").ap()
    eyebd_d = nc.dram_tensor("eyebd", [P, 128], f32, kind="ExternalInput").ap()
    diagm2_d = nc.dram_tensor("diagm2", [P, RL], bf16, kind="ExternalInput").ap()
    ones_d = nc.dram_tensor("ones125", [P, 1], bf16, kind="ExternalInput").ap()
    outT = nc.dram_tensor("outT", [ROWS_PER_CORE * L, D], bf16,
                          kind="ExternalOutput").ap()

    with tile.TileContext(nc) as tc:
        with (
            tc.tile_pool(name="const", bufs=1) as const,
            tc.tile_pool(name="xpool", bufs=6) as xpool,
            tc.tile_pool(name="hpool", bufs=3) as hpool,
            tc.tile_pool(name="spool", bufs=2) as spool,
            tc.tile_pool(name="ps_ft", bufs=3, space=bass.MemorySpace.PSUM) as ps_ft,
            tc.tile_pool(name="ps_sc", bufs=2, space=bass.MemorySpace.PSUM) as ps_sc,
            tc.tile_pool(name="ps_bt", bufs=1, space=bass.MemorySpace.PSUM) as ps_bt,
            tc.tile_pool(name="ps_h", bufs=2, space=bass.MemorySpace.PSUM) as ps_h,
        ):
            mtbd = const.tile([P, 128], bf16)
            nc.sync.dma_start(mtbd[:], mtbd_d[:])
            mbd = const.tile([P, 128], bf16)
            nc.sync.dma_start(mbd[:], mbd_d[:])
            eyebd = const.tile([P, 128], f32)
            nc.sync.dma_start(eyebd[:], eyebd_d[:])
            diagm2 = const.tile([P, RL], bf16)
            nc.sync.dma_start(diagm2[:], diagm2_d[:])
            ones125 = const.tile([P, 1], bf16)
            nc.sync.dma_start(ones125[:], ones_d[:])
            epsb = const.tile([P, 1], f32)
            nc.vector.memset(epsb[:], EPS)
            junk = const.tile([P, P], f32)

            # two persistent vt tiles (ping-pong); qw block written once
            vt_a = const.tile([128, NCHUNK * CW], bf16)
            vt_b = const.tile([128, NCHUNK * CW], bf16)
            vts = [vt_a, vt_b]
            for v in vts:
                nc.sync.dma_start(
                    v.rearrange("p (c w) -> p c w", w=CW)[:, :, 128 : 128 + L],
                    qwT_d.rearrange("p (c w) -> p c w", w=L),
                )

            starts = _batch_starts()
            X_tiles = {}

            def load(j):
                if j >= len(starts):
                    return
                row0 = starts[j]
                X = xpool.tile([P, D], bf16, name=f"X{j}", tag="X")
                nc.gpsimd.dma_start(X[:], loT[row0 * NJ : row0 * NJ + P, :])
                X_tiles[j] = X

            def ft_part(i):
                X = X_tiles[i]
                vt3 = vts[i % 2].rearrange("p (c w) -> p c w", w=CW)
                for half in range(4):
                    ftp = ps_ft.tile([128, 512], f32, name=f"ftp{i}_{half}",
                                     tag="ft")
                    for cc in range(4):
                        c = 4 * half + cc
                        nc.tensor.matmul(
                            ftp[:, 128 * cc : 128 * (cc + 1)],
                            X[:, 128 * c : 128 * (c + 1)],
                            mtbd[:],
                            start=True,
                            stop=True,
                        )
                    ft4 = ftp.rearrange("p (cc w) -> p cc w", w=128)
                    dst = vt3[:, 4 * half : 4 * half + 4, 0:128]
                    if half % 2 == 0:
                        nc.scalar.copy(dst, ft4)
                    else:
                        nc.vector.tensor_copy(dst, ft4)

            def sc_part(i):
                vt3 = vts[i % 2].rearrange("p (c w) -> p c w", w=CW)
                SCp = ps_sc.tile([128, CW], f32, name=f"SCp{i}", tag="sc")
                for c in range(NCHUNK):
                    nc.tensor.matmul(
                        SCp[0:P, :],
                        vt3[:, c, 0:P],
                        vt3[:, c, 0:CW],
                        start=(c == 0),
                        stop=(c == NCHUNK - 1),
                    )
                return SCp

            def scores_part(i, SCp):
                ssq = spool.tile([P, 1], f32, name=f"ssq{i}", tag="ssq")
                nc.vector.scalar_tensor_tensor(
                    out=junk[:, 0:P],
                    in0=SCp[0:P, 0:P],
                    scalar=1.0,
                    in1=eyebd[:, 0:P],
                    op0=AluOpType.mult,
                    op1=AluOpType.mult,
                    accum_out=ssq[:],
                )
                lnu = spool.tile([P, 1], f32, name=f"lnu{i}", tag="lnu")
                nc.scalar.activation(
                    lnu[:], ssq[:], mybir.ActivationFunctionType.Ln,
                    bias=epsb[:], scale=1.0 / D,
                )
                rsq = spool.tile([P, 1], f32, name=f"rsq{i}", tag="rsq")
                nc.scalar.activation(
                    rsq[:], lnu[:], mybir.ActivationFunctionType.Exp, scale=-0.5
                )
                exp_s = spool.tile([P, L], bf16, name=f"exps{i}", tag="exps")
                nc.scalar.activation(
                    exp_s[:], SCp[0:P, 128 : 128 + L],
                    mybir.ActivationFunctionType.Exp, scale=rsq[:],
                )
                esc2 = spool.tile([P, RL], bf16, name=f"esc2{i}", tag="esc2")
                nc.gpsimd.tensor_tensor(
                    esc2.rearrange("p (r l) -> p r l", r=R),
                    exp_s.unsqueeze(1).broadcast_to([P, R, L]),
                    diagm2.rearrange("p (r l) -> p r l", r=R),
                    AluOpType.mult,
                )
                return esc2

            def bt_part(i, esc2):
                bt = ps_bt.tile([128, 128], f32, name=f"bt{i}", tag="bt")
                nc.tensor.matmul(bt[:, 0:RL], mbd[:], esc2[:],
                                 start=True, stop=True)
                nc.tensor.matmul(bt[0:RL, 120:121], esc2[:], ones125[:],
                                 start=True, stop=True)
                rec = spool.tile([RL, 1], f32, name=f"rec{i}", tag="rec")
                nc.vector.reciprocal(rec[:], bt[0:RL, 120:121])
                btsb = spool.tile([P, RL], bf16, name=f"btsb{i}", tag="btsb")
                nc.vector.tensor_copy(btsb[:], bt[0:P, 0:RL])
                return dict(rec=rec, btsb=btsb)

            def h_part(i, row0, bts):
                X, rec, btsb = X_tiles.pop(i), bts["rec"], bts["btsb"]
                h_sb = hpool.tile([RL, D], bf16, name=f"hsb{i}", tag="h")
                for nb in range(4):
                    Hp = ps_h.tile([128, 512], f32, name=f"Hp{i}_{nb}", tag="hp")
                    nc.tensor.matmul(
                        Hp[0:RL, :],
                        btsb[:],
                        X[:, 512 * nb : 512 * (nb + 1)],
                        start=True,
                        stop=True,
                    )
                    dst = h_sb[:, 512 * nb : 512 * (nb + 1)]
                    if nb < 2:
                        nc.scalar.activation(
                            dst, Hp[0:RL, :],
                            mybir.ActivationFunctionType.Copy, scale=rec[:],
                        )
                    else:
                        nc.vector.scalar_tensor_tensor(
                            out=dst,
                            in0=Hp[0:RL, :],
                            scalar=1.0,
                            in1=rec.broadcast_to([RL, 512]),
                            op0=AluOpType.mult,
                            op1=AluOpType.mult,
                        )
                nc.scalar.dma_start(
                    outT[row0 * L : row0 * L + RL, :], h_sb[:]
                )

            PF = 3  # X prefetch depth
            for j in range(PF):
                load(j)
            prev = None  # (i, esc2)
            for i, row0 in enumerate(starts):
                ft_part(i)
                load(i + PF)
                if prev is not None:
                    bts = bt_part(prev[0], prev[1])
                SCp = sc_part(i)
                if prev is not None:
                    h_part(prev[0], starts[prev[0]], bts)
                esc2 = scores_part(i, SCp)
                prev = (i, esc2)
            bts = bt_part(prev[0], prev[1])
            h_part(prev[0], starts[prev[0]], bts)

    real_gat = bacc.get_activation_tables
    AF = mybir.ActivationFunctionType

    def gat_pinned(arch):
        out = {}
        for name, fns in real_gat(arch).items():
            if name == "natural_log_exp_and_others":
                out[name] = set(fns)
            else:
                out[name] = {f for f in fns if f not in (AF.Ln, AF.Exp)}
        return out

    bacc.get_activation_tables = gat_pinned
    try:
        if do_compile:
            nc.compile()
    finally:
        bacc.get_activation_tables = real_gat
    return nc


_NC_CACHE = None


def _prep_loT(layer_outputs, embedding):
    loT = np.empty((B * T, NJ, D), dtype=BF)
    loT[:, 0, :] = embedding.reshape(B * T, D).astype(BF)
    loT[:, 1:, :] = (
        layer_outputs.reshape(L, B * T, D).transpose(1, 0, 2).astype(BF)
    )
    return loT


def _make_in_maps(layer_outputs, embedding, queries, key_norm_weight):
    loT = _prep_loT(layer_outputs, embedding)
    consts = _build_consts(queries, key_norm_weight)
    in_maps = []
    for c in range(N_CORES):
        r0 = c * ROWS_PER_CORE
        in_maps.append({
            "loT": loT[r0 : r0 + ROWS_PER_CORE].reshape(ROWS_PER_CORE * NJ, D),
            "qwT": consts["qwT"],
            "mtbd": consts["mtbd"],
            "mbd": consts["mbd"],
            "eyebd": consts["eyebd"],
            "diagm2": consts["diagm2"],
            "ones125": consts["ones125"],
        })
    return in_maps


def kernel(layer_outputs, embedding, queries, key_norm_weight):
    global _NC_CACHE
    layer_outputs = np.asarray(layer_outputs, dtype=np.float32)
    embedding = np.asarray(embedding, dtype=np.float32)
    queries = np.asarray(queries, dtype=np.float32)
    key_norm_weight = np.asarray(key_norm_weight, dtype=np.float32)

    in_maps = _make_in_maps(layer_outputs, embedding, queries, key_norm_weight)

    if _NC_CACHE is None:
        _NC_CACHE = build_kernel()
    nc = _NC_CACHE

    res = run_bass_kernel_spmd(nc, in_maps, core_ids=list(range(N_CORES)))

    full = np.empty((L, B * T, D), dtype=np.float32)
    for c in range(N_CORES):
        r0 = c * ROWS_PER_CORE
        outT = res.results[c]["outT"].astype(np.float32).reshape(
            ROWS_PER_CORE, L, D
        )
        full[:, r0 : r0 + ROWS_PER_CORE, :] = outT.transpose(1, 0, 2)
    return full.reshape(L, B, T, D)


# revision 21
# speedup vs baseline: 1.2147x; 1.0176x over previous
"""Trainium2 Bass kernel for BlockAttnRes.compute_all_inputs (bf16 pipeline).

v3: lean softmax path (no PE transposes, no mask-add), 1/ssum folded into
H copies, copies spread over scalar/vector/gpsimd, software-pipelined
emission. Input DMA on SP, output on scalar HWDGE.
"""

import numpy as np
import ml_dtypes

import concourse.bass as bass
import concourse.bacc as bacc
import concourse.mybir as mybir
from concourse import tile
from concourse.alu_op_type import AluOpType
from concourse.bass_utils import run_bass_kernel_spmd

L = 24
D = 2048
NUM_BLOCKS = 8
EPS = 1e-6
B, T = 2, 1024
N_CORES = 8

ROWS_PER_CORE = (B * T) // N_CORES  # 256
R = 5              # rows per batch
NJ = 25            # raw vectors per row: emb + 24 layer outputs
NS = 25            # sources per row
P = NJ * R         # 125 partitions per batch
RL = R * L         # 120
NCHUNK = D // 128  # 16 d-chunks
CW = 152           # vt per-chunk pitch: 128 (VT, cols 125:128 zero) + 24 qwT

f32 = mybir.dt.float32
bf16 = mybir.dt.bfloat16
BF = ml_dtypes.bfloat16


def _source_matrix():
    M = np.zeros((NS, NJ), dtype=np.float32)
    M[0, 0] = 1.0
    for k in range(NUM_BLOCKS):
        for i in range(3):
            M[1 + 3 * k + i, 1 + 3 * k : 1 + 3 * k + i + 1] = 1.0
    return M


def _valid_matrix():
    V = np.zeros((L, NS), dtype=bool)
    for l in range(L):
        kb, ii = l // 3, l % 3
        V[l, 0] = True
        for k in range(kb):
            V[l, 3 * k + 3] = True
        if ii > 0:
            V[l, 3 * kb + ii] = True
    return V


def _build_consts(queries, key_norm_weight):
    M = _source_matrix()
    valid = _valid_matrix()
    eye_r = np.eye(R, dtype=np.float32)

    qw = (queries * key_norm_weight[None, :]).astype(np.float32)  # [L, D]
    qwT = np.ascontiguousarray(
        qw.reshape(L, NCHUNK, 128).transpose(2, 1, 0).reshape(128, NCHUNK * L)
    ).astype(BF)

    # mtbd[(a,j), (b,n)] = M[n,j] * (a==b);  rows a*NJ+j, cols b*NS+n
    mtbd = np.einsum("nj,ab->ajbn", M, eye_r).reshape(P, NS * R)
    mtbd128 = np.zeros((P, 128), np.float32)
    mtbd128[:, :P] = mtbd
    mtbd128 = mtbd128.astype(BF)
    # mbd[(a,n), (b,j)] = M[n,j] * (a==b);  rows a*NS+n, cols b*NJ+j
    mbd = np.einsum("nj,ab->anbj", M, eye_r).reshape(NS * R, P)
    mbd128 = np.zeros((P, 128), np.float32)
    mbd128[:, :P] = mbd
    mbd128 = mbd128.astype(BF)
    # eyebd for diag extraction of the source gram
    eye_bd = np.zeros((P, 128), np.float32)
    eye_bd[:, :P] = np.eye(P, dtype=np.float32)
    # diagm2[(b,n), (r,l)] = (b==r) * valid[l, n]
    diagm2 = np.zeros((P, RL), np.float32)
    for b in range(R):
        for n in range(NS):
            for l in range(L):
                if valid[l, n]:
                    diagm2[b * NS + n, b * L + l] = 1.0
    diagm2 = diagm2.astype(BF)
    ones125 = np.ones((P, 1), np.float32).astype(BF)
    return dict(qwT=qwT, mtbd=mtbd128, mbd=mbd128, eyebd=eye_bd,
                diagm2=diagm2, ones125=ones125)


def _batch_starts():
    starts = [R * b for b in range(ROWS_PER_CORE // R)]  # 0..250
    if starts[-1] + R < ROWS_PER_CORE:
        starts.append(ROWS_PER_CORE - R)  # 251 (overlaps; identical rewrites)
    return starts


def build_kernel(do_compile=True):
    nc = bacc.Bacc("TRN2", target_bir_lowering=False, debug=False)

    loT = nc.dram_tensor("loT", [ROWS_PER_CORE * NJ, D], bf16,
                         kind="ExternalInput").ap()
    qwT_d = nc.dram_tensor("qwT", [128, NCHUNK * L], bf16, kind="ExternalInput").ap()
    mtbd_d = nc.dram_tensor("mtbd", [P, 128], bf16, kind="ExternalInput").ap()
    mbd_d = nc.dram_tensor("mbd", [P, 128], bf16, kind="ExternalInput").ap()
    eyebd_d = nc.dram_tensor("eyebd", [P, 128], f32, kind="ExternalInput").ap()
    diagm2_d = nc.dram_tensor("diagm2", [P, RL], bf16, kind="ExternalInput").ap()
    ones_d = nc.dram_tensor("ones125", [P, 1], bf16, kind="ExternalInput").ap()
    outT = nc.dram_tensor("outT", [ROWS_PER_CORE * L, D], bf16,
                          kind="ExternalOutput").ap()

    with tile.TileContext(nc) as tc:
        with (
            tc.tile_pool(name="const", bufs=1) as const,
            tc.tile_pool(name="xpool", bufs=6) as xpool,
            tc.tile_pool(name="hpool", bufs=3) as hpool,
            tc.tile_pool(name="spool", bufs=2) as spool,
            tc.tile_pool(name="ps_ft", bufs=2, space=bass.MemorySpace.PSUM) as ps_ft,
            tc.tile_pool(name="ps_sc", bufs=2, space=bass.MemorySpace.PSUM) as ps_sc,
            tc.tile_pool(name="ps_bt", bufs=2, space=bass.MemorySpace.PSUM) as ps_bt,
            tc.tile_pool(name="ps_h", bufs=2, space=bass.MemorySpace.PSUM) as ps_h,
        ):
            mtbd = const.tile([P, 128], bf16)
            nc.sync.dma_start(mtbd[:], mtbd_d[:])
            mbd = const.tile([P, 128], bf16)
            nc.sync.dma_start(mbd[:], mbd_d[:])
            eyebd = const.tile([P, 128], f32)
            nc.sync.dma_start(eyebd[:], eyebd_d[:])
            diagm2 = const.tile([P, RL], bf16)
            nc.sync.dma_start(diagm2[:], diagm2_d[:])
            ones125 = const.tile([P, 1], bf16)
            nc.sync.dma_start(ones125[:], ones_d[:])
            epsb = const.tile([P, 1], f32)
            nc.vector.memset(epsb[:], EPS)
            junk = const.tile([P, P], f32)

            # two persistent vt tiles (ping-pong); qw block written once
            vt_a = const.tile([128, NCHUNK * CW], bf16)
            vt_b = const.tile([128, NCHUNK * CW], bf16)
            vts = [vt_a, vt_b]
            for v in vts:
                nc.sync.dma_start(
                    v.rearrange("p (c w) -> p c w", w=CW)[:, :, 128 : 128 + L],
                    qwT_d.rearrange("p (c w) -> p c w", w=L),
                )

            starts = _batch_starts()
            X_tiles = {}

            def load(j):
                if j >= len(starts):
                    return
                row0 = starts[j]
                X = xpool.tile([P, D], bf16, name=f"X{j}", tag="X")
                nc.gpsimd.dma_start(X[:], loT[row0 * NJ : row0 * NJ + P, :])
                X_tiles[j] = X

            def ft_part(i):
                X = X_tiles[i]
                vt3 = vts[i % 2].rearrange("p (c w) -> p c w", w=CW)
                for half in range(4):
                    ftp = ps_ft.tile([128, 512], f32, name=f"ftp{i}_{half}",
                                     tag="ft")
                    for cc in range(4):
                        c = 4 * half + cc
                        nc.tensor.matmul(
                            ftp[:, 128 * cc : 128 * (cc + 1)],
                            X[:, 128 * c : 128 * (c + 1)],
                            mtbd[:],
                            start=True,
                            stop=True,
                        )
                    ft4 = ftp.rearrange("p (cc w) -> p cc w", w=128)
                    dst = vt3[:, 4 * half : 4 * half + 4, 0:128]
                    if half % 2 == 0:
                        nc.scalar.copy(dst, ft4)
                    else:
                        nc.vector.tensor_copy(dst, ft4)

            def sc_part(i):
                vt3 = vts[i % 2].rearrange("p (c w) -> p c w", w=CW)
                SCp = ps_sc.tile([128, CW], f32, name=f"SCp{i}", tag="sc")
                for c in range(NCHUNK):
                    nc.tensor.matmul(
                        SCp[0:P, :],
                        vt3[:, c, 0:P],
                        vt3[:, c, 0:CW],
                        start=(c == 0),
                        stop=(c == NCHUNK - 1),
                    )
                return SCp

            def scores_part(i, SCp):
                ssq = spool.tile([P, 1], f32, name=f"ssq{i}", tag="ssq")
                nc.vector.scalar_tensor_tensor(
                    out=junk[:, 0:P],
                    in0=SCp[0:P, 0:P],
                    scalar=1.0,
                    in1=eyebd[:, 0:P],
                    op0=AluOpType.mult,
                    op1=AluOpType.mult,
                    accum_out=ssq[:],
                )
                lnu = spool.tile([P, 1], f32, name=f"lnu{i}", tag="lnu")
                nc.scalar.activation(
                    lnu[:], ssq[:], mybir.ActivationFunctionType.Ln,
                    bias=epsb[:], scale=1.0 / D,
                )
                rsq = spool.tile([P, 1], f32, name=f"rsq{i}", tag="rsq")
                nc.scalar.activation(
                    rsq[:], lnu[:], mybir.ActivationFunctionType.Exp, scale=-0.5
                )
                exp_s = spool.tile([P, L], bf16, name=f"exps{i}", tag="exps")
                nc.scalar.activation(
                    exp_s[:], SCp[0:P, 128 : 128 + L],
                    mybir.ActivationFunctionType.Exp, scale=rsq[:],
                )
                esc2 = spool.tile([P, RL], bf16, name=f"esc2{i}", tag="esc2")
                nc.gpsimd.tensor_tensor(
                    esc2.rearrange("p (r l) -> p r l", r=R),
                    exp_s.unsqueeze(1).broadcast_to([P, R, L]),
                    diagm2.rearrange("p (r l) -> p r l", r=R),
                    AluOpType.mult,
                )
                return esc2

            def bt_part(i, esc2):
                bt = ps_bt.tile([128, 128], f32, name=f"bt{i}", tag="bt")
                nc.tensor.matmul(bt[:, 0:RL], mbd[:], esc2[:],
                                 start=True, stop=True)
                nc.tensor.matmul(bt[0:RL, 120:121], esc2[:], ones125[:],
                                 start=True, stop=True)
                rec = spool.tile([RL, 1], f32, name=f"rec{i}", tag="rec")
                nc.vector.reciprocal(rec[:], bt[0:RL, 120:121])
                btsb = spool.tile([P, RL], bf16, name=f"btsb{i}", tag="btsb")
                nc.vector.tensor_copy(btsb[:], bt[0:P, 0:RL])
                return dict(rec=rec, btsb=btsb)

            def h_part(i, row0, bts):
                X, rec, btsb = X_tiles.pop(i), bts["rec"], bts["btsb"]
                h_sb = hpool.tile([RL, D], bf16, name=f"hsb{i}", tag="h")
                for nb in range(4):
                    Hp = ps_h.tile([128, 512], f32, name=f"Hp{i}_{nb}", tag="hp")
                    nc.tensor.matmul(
                        Hp[0:RL, :],
                        btsb[:],
                        X[:, 512 * nb : 512 * (nb + 1)],
                        start=True,
                        stop=True,
                    )
                    dst = h_sb[:, 512 * nb : 512 * (nb + 1)]
                    if nb < 2:
                        nc.scalar.activation(
                            dst, Hp[0:RL, :],
                            mybir.ActivationFunctionType.Copy, scale=rec[:],
                        )
                    else:
                        nc.vector.scalar_tensor_tensor(
                            out=dst,
                            in0=Hp[0:RL, :],
                            scalar=1.0,
                            in1=rec.broadcast_to([RL, 512]),
                            op0=AluOpType.mult,
                            op1=AluOpType.mult,
                        )
                nc.scalar.dma_start(
                    outT[row0 * L : row0 * L + RL, :], h_sb[:]
                )

            PF = 3  # X prefetch depth
            for j in range(PF):
                load(j)
            prev = None  # (i, esc2)
            for i, row0 in enumerate(starts):
                ft_part(i)
                load(i + PF)
                if prev is not None:
                    bts = bt_part(prev[0], prev[1])
                SCp = sc_part(i)
                if prev is not None:
                    h_part(prev[0], starts[prev[0]], bts)
                esc2 = scores_part(i, SCp)
                prev = (i, esc2)
            bts = bt_part(prev[0], prev[1])
            h_part(prev[0], starts[prev[0]], bts)

    real_gat = bacc.get_activation_tables
    AF = mybir.ActivationFunctionType

    def gat_pinned(arch):
        out = {}
        for name, fns in real_gat(arch).items():
            if name == "natural_log_exp_and_others":
                out[name] = set(fns)
            else:
                out[name] = {f for f in fns if f not in (AF.Ln, AF.Exp)}
        return out

    bacc.get_activation_tables = gat_pinned
    try:
        if do_compile:
            nc.compile()
    finally:
        bacc.get_activation_tables = real_gat
    return nc


_NC_CACHE = None


def _prep_loT(layer_outputs, embedding):
    loT = np.empty((B * T, NJ, D), dtype=BF)
    loT[:, 0, :] = embedding.reshape(B * T, D).astype(BF)
    loT[:, 1:, :] = (
        layer_outputs.reshape(L, B * T, D).transpose(1, 0, 2).astype(BF)
    )
    return loT


def _make_in_maps(layer_outputs, embedding, queries, key_norm_weight):
    loT = _prep_loT(layer_outputs, embedding)
    consts = _build_consts(queries, key_norm_weight)
    in_maps = []
    for c in range(N_CORES):
        r0 = c * ROWS_PER_CORE
        in_maps.append({
            "loT": loT[r0 : r0 + ROWS_PER_CORE].reshape(ROWS_PER_CORE * NJ, D),
            "qwT": consts["qwT"],
            "mtbd": consts["mtbd"],
            "mbd": consts["mbd"],
            "eyebd": consts["eyebd"],
            "diagm2": consts["diagm2"],
            "ones125": consts["ones125"],
        })
    return in_maps


def kernel(layer_outputs, embedding, queries, key_norm_weight):
    global _NC_CACHE
    layer_outputs = np.asarray(layer_outputs, dtype=np.float32)
    embedding = np.asarray(embedding, dtype=np.float32)
    queries = np.asarray(queries, dtype=np.float32)
    key_norm_weight = np.asarray(key_norm_weight, dtype=np.float32)

    in_maps = _make_in_maps(layer_outputs, embedding, queries, key_norm_weight)

    if _NC_CACHE is None:
        _NC_CACHE = build_kernel()
    nc = _NC_CACHE

    res = run_bass_kernel_spmd(nc, in_maps, core_ids=list(range(N_CORES)))

    full = np.empty((L, B * T, D), dtype=np.float32)
    for c in range(N_CORES):
        r0 = c * ROWS_PER_CORE
        outT = res.results[c]["outT"].astype(np.float32).reshape(
            ROWS_PER_CORE, L, D
        )
        full[:, r0 : r0 + ROWS_PER_CORE, :] = outT.transpose(1, 0, 2)
    return full.reshape(L, B, T, D)


# revision 22
# speedup vs baseline: 1.2206x; 1.0049x over previous
"""Trainium2 Bass kernel for BlockAttnRes.compute_all_inputs (bf16 pipeline).

v3: lean softmax path (no PE transposes, no mask-add), 1/ssum folded into
H copies, copies spread over scalar/vector/gpsimd, software-pipelined
emission. Input DMA on SP, output on scalar HWDGE.
"""

import numpy as np
import ml_dtypes

import concourse.bass as bass
import concourse.bacc as bacc
import concourse.mybir as mybir
from concourse import tile
from concourse.alu_op_type import AluOpType
from concourse.bass_utils import run_bass_kernel_spmd

L = 24
D = 2048
NUM_BLOCKS = 8
EPS = 1e-6
B, T = 2, 1024
N_CORES = 8

ROWS_PER_CORE = (B * T) // N_CORES  # 256
R = 5              # rows per batch
NJ = 25            # raw vectors per row: emb + 24 layer outputs
NS = 25            # sources per row
P = NJ * R         # 125 partitions per batch
RL = R * L         # 120
NCHUNK = D // 128  # 16 d-chunks
CW = 152           # vt per-chunk pitch: 128 (VT, cols 125:128 zero) + 24 qwT

f32 = mybir.dt.float32
bf16 = mybir.dt.bfloat16
BF = ml_dtypes.bfloat16


def _source_matrix():
    M = np.zeros((NS, NJ), dtype=np.float32)
    M[0, 0] = 1.0
    for k in range(NUM_BLOCKS):
        for i in range(3):
            M[1 + 3 * k + i, 1 + 3 * k : 1 + 3 * k + i + 1] = 1.0
    return M


def _valid_matrix():
    V = np.zeros((L, NS), dtype=bool)
    for l in range(L):
        kb, ii = l // 3, l % 3
        V[l, 0] = True
        for k in range(kb):
            V[l, 3 * k + 3] = True
        if ii > 0:
            V[l, 3 * kb + ii] = True
    return V


def _build_consts(queries, key_norm_weight):
    M = _source_matrix()
    valid = _valid_matrix()
    eye_r = np.eye(R, dtype=np.float32)

    qw = (queries * key_norm_weight[None, :]).astype(np.float32)  # [L, D]
    qwT = np.ascontiguousarray(
        qw.reshape(L, NCHUNK, 128).transpose(2, 1, 0).reshape(128, NCHUNK * L)
    ).astype(BF)

    # mtbd[(a,j), (b,n)] = M[n,j] * (a==b);  rows a*NJ+j, cols b*NS+n
    mtbd = np.einsum("nj,ab->ajbn", M, eye_r).reshape(P, NS * R)
    mtbd128 = np.zeros((P, 128), np.float32)
    mtbd128[:, :P] = mtbd
    mtbd128 = mtbd128.astype(BF)
    # mbd[(a,n), (b,j)] = M[n,j] * (a==b);  rows a*NS+n, cols b*NJ+j
    mbd = np.einsum("nj,ab->anbj", M, eye_r).reshape(NS * R, P)
    mbd128 = np.zeros((P, 128), np.float32)
    mbd128[:, :P] = mbd
    mbd128 = mbd128.astype(BF)
    # eyebd for diag extraction of the source gram
    eye_bd = np.zeros((P, 128), np.float32)
    eye_bd[:, :P] = np.eye(P, dtype=np.float32)
    # diagm2[(b,n), (r,l)] = (b==r) * valid[l, n]
    diagm2 = np.zeros((P, RL), np.float32)
    for b in range(R):
        for n in range(NS):
            for l in range(L):
                if valid[l, n]:
                    diagm2[b * NS + n, b * L + l] = 1.0
    diagm2 = diagm2.astype(BF)
    ones125 = np.ones((P, 1), np.float32).astype(BF)
    return dict(qwT=qwT, mtbd=mtbd128, mbd=mbd128, eyebd=eye_bd,
                diagm2=diagm2, ones125=ones125)


def _batch_starts():
    starts = [R * b for b in range(ROWS_PER_CORE // R)]  # 0..250
    if starts[-1] + R < ROWS_PER_CORE:
        starts.append(ROWS_PER_CORE - R)  # 251 (overlaps; identical rewrites)
    return starts


def build_kernel(do_compile=True):
    nc = bacc.Bacc("TRN2", target_bir_lowering=False, debug=False)

    loT = nc.dram_tensor("loT", [ROWS_PER_CORE * NJ, D], bf16,
                         kind="ExternalInput").ap()
    qwT_d = nc.dram_tensor("qwT", [128, NCHUNK * L], bf16, kind="ExternalInput").ap()
    mtbd_d = nc.dram_tensor("mtbd", [P, 128], bf16, kind="ExternalInput").ap()
    mbd_d = nc.dram_tensor("mbd", [P, 128], bf16, kind="ExternalInput").ap()
    eyebd_d = nc.dram_tensor("eyebd", [P, 128], f32, kind="ExternalInput").ap()
    diagm2_d = nc.dram_tensor("diagm2", [P, RL], bf16, kind="ExternalInput").ap()
    ones_d = nc.dram_tensor("ones125", [P, 1], bf16, kind="ExternalInput").ap()
    outT = nc.dram_tensor("outT", [ROWS_PER_CORE * L, D], bf16,
                          kind="ExternalOutput").ap()

    with tile.TileContext(nc) as tc:
        with (
            tc.tile_pool(name="const", bufs=1) as const,
            tc.tile_pool(name="xpool", bufs=6) as xpool,
            tc.tile_pool(name="hpool", bufs=3) as hpool,
            tc.tile_pool(name="spool", bufs=2) as spool,
            tc.tile_pool(name="ps_ft", bufs=3, space=bass.MemorySpace.PSUM) as ps_ft,
            tc.tile_pool(name="ps_sc", bufs=1, space=bass.MemorySpace.PSUM) as ps_sc,
            tc.tile_pool(name="ps_bt", bufs=2, space=bass.MemorySpace.PSUM) as ps_bt,
            tc.tile_pool(name="ps_h", bufs=2, space=bass.MemorySpace.PSUM) as ps_h,
        ):
            mtbd = const.tile([P, 128], bf16)
            nc.sync.dma_start(mtbd[:], mtbd_d[:])
            mbd = const.tile([P, 128], bf16)
            nc.sync.dma_start(mbd[:], mbd_d[:])
            eyebd = const.tile([P, 128], f32)
            nc.sync.dma_start(eyebd[:], eyebd_d[:])
            diagm2 = const.tile([P, RL], bf16)
            nc.sync.dma_start(diagm2[:], diagm2_d[:])
            ones125 = const.tile([P, 1], bf16)
            nc.sync.dma_start(ones125[:], ones_d[:])
            epsb = const.tile([P, 1], f32)
            nc.vector.memset(epsb[:], EPS)
            junk = const.tile([P, P], f32)

            # two persistent vt tiles (ping-pong); qw block written once
            vt_a = const.tile([128, NCHUNK * CW], bf16)
            vt_b = const.tile([128, NCHUNK * CW], bf16)
            vts = [vt_a, vt_b]
            for v in vts:
                nc.sync.dma_start(
                    v.rearrange("p (c w) -> p c w", w=CW)[:, :, 128 : 128 + L],
                    qwT_d.rearrange("p (c w) -> p c w", w=L),
                )

            starts = _batch_starts()
            X_tiles = {}

            def load(j):
                if j >= len(starts):
                    return
                row0 = starts[j]
                X = xpool.tile([P, D], bf16, name=f"X{j}", tag="X")
                nc.gpsimd.dma_start(X[:], loT[row0 * NJ : row0 * NJ + P, :])
                X_tiles[j] = X

            def ft_part(i):
                X = X_tiles[i]
                vt3 = vts[i % 2].rearrange("p (c w) -> p c w", w=CW)
                for half in range(4):
                    ftp = ps_ft.tile([128, 512], f32, name=f"ftp{i}_{half}",
                                     tag="ft")
                    for cc in range(4):
                        c = 4 * half + cc
                        nc.tensor.matmul(
                            ftp[:, 128 * cc : 128 * (cc + 1)],
                            X[:, 128 * c : 128 * (c + 1)],
                            mtbd[:],
                            start=True,
                            stop=True,
                        )
                    ft4 = ftp.rearrange("p (cc w) -> p cc w", w=128)
                    dst = vt3[:, 4 * half : 4 * half + 4, 0:128]
                    if half % 2 == 0:
                        nc.scalar.copy(dst, ft4)
                    else:
                        nc.vector.tensor_copy(dst, ft4)

            def sc_part(i):
                vt3 = vts[i % 2].rearrange("p (c w) -> p c w", w=CW)
                SCp = ps_sc.tile([128, CW], f32, name=f"SCp{i}", tag="sc")
                for c in range(NCHUNK):
                    nc.tensor.matmul(
                        SCp[0:P, :],
                        vt3[:, c, 0:P],
                        vt3[:, c, 0:CW],
                        start=(c == 0),
                        stop=(c == NCHUNK - 1),
                    )
                return SCp

            def scores_part(i, SCp):
                ssq = spool.tile([P, 1], f32, name=f"ssq{i}", tag="ssq")
                nc.vector.scalar_tensor_tensor(
                    out=junk[:, 0:P],
                    in0=SCp[0:P, 0:P],
                    scalar=1.0,
                    in1=eyebd[:, 0:P],
                    op0=AluOpType.mult,
                    op1=AluOpType.mult,
                    accum_out=ssq[:],
                )
                lnu = spool.tile([P, 1], f32, name=f"lnu{i}", tag="lnu")
                nc.scalar.activation(
                    lnu[:], ssq[:], mybir.ActivationFunctionType.Ln,
                    bias=epsb[:], scale=1.0 / D,
                )
                rsq = spool.tile([P, 1], f32, name=f"rsq{i}", tag="rsq")
                nc.scalar.activation(
                    rsq[:], lnu[:], mybir.ActivationFunctionType.Exp, scale=-0.5
                )
                exp_s = spool.tile([P, L], bf16, name=f"exps{i}", tag="exps")
                nc.scalar.activation(
                    exp_s[:], SCp[0:P, 128 : 128 + L],
                    mybir.ActivationFunctionType.Exp, scale=rsq[:],
                )
                esc2 = spool.tile([P, RL], bf16, name=f"esc2{i}", tag="esc2")
                nc.gpsimd.tensor_tensor(
                    esc2.rearrange("p (r l) -> p r l", r=R),
                    exp_s.unsqueeze(1).broadcast_to([P, R, L]),
                    diagm2.rearrange("p (r l) -> p r l", r=R),
                    AluOpType.mult,
                )
                return esc2

            def bt_part(i, esc2):
                bt = ps_bt.tile([128, 128], f32, name=f"bt{i}", tag="bt")
                nc.tensor.matmul(bt[:, 0:RL], mbd[:], esc2[:],
                                 start=True, stop=True)
                nc.tensor.matmul(bt[0:RL, 120:121], esc2[:], ones125[:],
                                 start=True, stop=True)
                rec = spool.tile([RL, 1], f32, name=f"rec{i}", tag="rec")
                nc.vector.reciprocal(rec[:], bt[0:RL, 120:121])
                btsb = spool.tile([P, RL], bf16, name=f"btsb{i}", tag="btsb")
                nc.vector.tensor_copy(btsb[:], bt[0:P, 0:RL])
                return dict(rec=rec, btsb=btsb)

            def h_part(i, row0, bts):
                X, rec, btsb = X_tiles.pop(i), bts["rec"], bts["btsb"]
                h_sb = hpool.tile([RL, D], bf16, name=f"hsb{i}", tag="h")
                for nb in range(4):
                    Hp = ps_h.tile([128, 512], f32, name=f"Hp{i}_{nb}", tag="hp")
                    nc.tensor.matmul(
                        Hp[0:RL, :],
                        btsb[:],
                        X[:, 512 * nb : 512 * (nb + 1)],
                        start=True,
                        stop=True,
                    )
                    dst = h_sb[:, 512 * nb : 512 * (nb + 1)]
                    if nb < 2:
                        nc.scalar.activation(
                            dst, Hp[0:RL, :],
                            mybir.ActivationFunctionType.Copy, scale=rec[:],
                        )
                    else:
                        nc.vector.scalar_tensor_tensor(
                            out=dst,
                            in0=Hp[0:RL, :],
                            scalar=1.0,
                            in1=rec.broadcast_to([RL, 512]),
                            op0=AluOpType.mult,
                            op1=AluOpType.mult,
                        )
                nc.scalar.dma_start(
                    outT[row0 * L : row0 * L + RL, :], h_sb[:]
                )

            PF = 3  # X prefetch depth
            for j in range(PF):
                load(j)
            prev = None  # (i, esc2)
            for i, row0 in enumerate(starts):
                ft_part(i)
                load(i + PF)
                if prev is not None:
                    bts = bt_part(prev[0], prev[1])
                SCp = sc_part(i)
                if prev is not None:
                    h_part(prev[0], starts[prev[0]], bts)
                esc2 = scores_part(i, SCp)
                prev = (i, esc2)
            bts = bt_part(prev[0], prev[1])
            h_part(prev[0], starts[prev[0]], bts)

    real_gat = bacc.get_activation_tables
    AF = mybir.ActivationFunctionType

    def gat_pinned(arch):
        out = {}
        for name, fns in real_gat(arch).items():
            if name == "natural_log_exp_and_others":
                out[name] = set(fns)
            else:
                out[name] = {f for f in fns if f not in (AF.Ln, AF.Exp)}
        return out

    bacc.get_activation_tables = gat_pinned
    try:
        if do_compile:
            nc.compile()
    finally:
        bacc.get_activation_tables = real_gat
    return nc


_NC_CACHE = None


def _prep_loT(layer_outputs, embedding):
    loT = np.empty((B * T, NJ, D), dtype=BF)
    loT[:, 0, :] = embedding.reshape(B * T, D).astype(BF)
    loT[:, 1:, :] = (
        layer_outputs.reshape(L, B * T, D).transpose(1, 0, 2).astype(BF)
    )
    return loT


def _make_in_maps(layer_outputs, embedding, queries, key_norm_weight):
    loT = _prep_loT(layer_outputs, embedding)
    consts = _build_consts(queries, key_norm_weight)
    in_maps = []
    for c in range(N_CORES):
        r0 = c * ROWS_PER_CORE
        in_maps.append({
            "loT": loT[r0 : r0 + ROWS_PER_CORE].reshape(ROWS_PER_CORE * NJ, D),
            "qwT": consts["qwT"],
            "mtbd": consts["mtbd"],
            "mbd": consts["mbd"],
            "eyebd": consts["eyebd"],
            "diagm2": consts["diagm2"],
            "ones125": consts["ones125"],
        })
    return in_maps


def kernel(layer_outputs, embedding, queries, key_norm_weight):
    global _NC_CACHE
    layer_outputs = np.asarray(layer_outputs, dtype=np.float32)
    embedding = np.asarray(embedding, dtype=np.float32)
    queries = np.asarray(queries, dtype=np.float32)
    key_norm_weight = np.asarray(key_norm_weight, dtype=np.float32)

    in_maps = _make_in_maps(layer_outputs, embedding, queries, key_norm_weight)

    if _NC_CACHE is None:
        _NC_CACHE = build_kernel()
    nc = _NC_CACHE

    res = run_bass_kernel_spmd(nc, in_maps, core_ids=list(range(N_CORES)))

    full = np.empty((L, B * T, D), dtype=np.float32)
    for c in range(N_CORES):
        r0 = c * ROWS_PER_CORE
        outT = res.results[c]["outT"].astype(np.float32).reshape(
            ROWS_PER_CORE, L, D
        )
        full[:, r0 : r0 + ROWS_PER_CORE, :] = outT.transpose(1, 0, 2)
    return full.reshape(L, B, T, D)


# revision 25
# speedup vs baseline: 1.4247x; 1.1673x over previous
"""Trainium2 Bass kernel for BlockAttnRes.compute_all_inputs (bf16 pipeline).

v3: lean softmax path (no PE transposes, no mask-add), 1/ssum folded into
H copies, copies spread over scalar/vector/gpsimd, software-pipelined
emission. Input DMA on SP, output on scalar HWDGE.
"""

import numpy as np
import ml_dtypes

import concourse.bass as bass
import concourse.bacc as bacc
import concourse.mybir as mybir
from concourse import tile
from concourse.alu_op_type import AluOpType
from concourse.bass_utils import run_bass_kernel_spmd

L = 24
D = 2048
NUM_BLOCKS = 8
EPS = 1e-6
B, T = 2, 1024
N_CORES = 8

ROWS_PER_CORE = (B * T) // N_CORES  # 256
R = 5              # rows per batch
NJ = 25            # raw vectors per row: emb + 24 layer outputs
NS = 25            # sources per row
P = NJ * R         # 125 partitions per batch
RL = R * L         # 120
NCHUNK = D // 128  # 16 d-chunks
CW = 152           # vt per-chunk pitch: 128 (VT, cols 125:128 zero) + 24 qwT

f32 = mybir.dt.float32
bf16 = mybir.dt.bfloat16
BF = ml_dtypes.bfloat16


def _source_matrix():
    M = np.zeros((NS, NJ), dtype=np.float32)
    M[0, 0] = 1.0
    for k in range(NUM_BLOCKS):
        for i in range(3):
            M[1 + 3 * k + i, 1 + 3 * k : 1 + 3 * k + i + 1] = 1.0
    return M


def _valid_matrix():
    V = np.zeros((L, NS), dtype=bool)
    for l in range(L):
        kb, ii = l // 3, l % 3
        V[l, 0] = True
        for k in range(kb):
            V[l, 3 * k + 3] = True
        if ii > 0:
            V[l, 3 * kb + ii] = True
    return V


def _build_consts(queries, key_norm_weight):
    M = _source_matrix()
    valid = _valid_matrix()
    eye_r = np.eye(R, dtype=np.float32)

    qw = (queries * key_norm_weight[None, :]).astype(np.float32)  # [L, D]
    qwT = np.ascontiguousarray(
        qw.reshape(L, NCHUNK, 128).transpose(2, 1, 0).reshape(128, NCHUNK * L)
    ).astype(BF)

    # mtbd[(a,j), (b,n)] = M[n,j] * (a==b);  rows a*NJ+j, cols b*NS+n
    mtbd = np.einsum("nj,ab->ajbn", M, eye_r).reshape(P, NS * R)
    mtbd128 = np.zeros((P, 128), np.float32)
    mtbd128[:, :P] = mtbd
    mtbd128 = mtbd128.astype(BF)
    # mbd[(a,n), (b,j)] = M[n,j] * (a==b);  rows a*NS+n, cols b*NJ+j
    mbd = np.einsum("nj,ab->anbj", M, eye_r).reshape(NS * R, P)
    mbd128 = np.zeros((P, 128), np.float32)
    mbd128[:, :P] = mbd
    mbd128 = mbd128.astype(BF)
    # eyebd for diag extraction of the source gram
    eye_bd = np.zeros((P, 128), np.float32)
    eye_bd[:, :P] = np.eye(P, dtype=np.float32)
    # diagm2[(b,n), (r,l)] = (b==r) * valid[l, n]
    diagm2 = np.zeros((P, RL), np.float32)
    for b in range(R):
        for n in range(NS):
            for l in range(L):
                if valid[l, n]:
                    diagm2[b * NS + n, b * L + l] = 1.0
    diagm2 = diagm2.astype(BF)
    ones125 = np.ones((P, 1), np.float32).astype(BF)
    return dict(qwT=qwT, mtbd=mtbd128, mbd=mbd128, eyebd=eye_bd,
                diagm2=diagm2, ones125=ones125)


def _batch_starts():
    starts = [R * b for b in range(ROWS_PER_CORE // R)]  # 0..250
    if starts[-1] + R < ROWS_PER_CORE:
        starts.append(ROWS_PER_CORE - R)  # 251 (overlaps; identical rewrites)
    return starts


def build_kernel(do_compile=True):
    nc = bacc.Bacc("TRN2", target_bir_lowering=False, debug=False)

    loT = nc.dram_tensor("loT", [ROWS_PER_CORE * NJ, D], bf16,
                         kind="ExternalInput").ap()
    qwT_d = nc.dram_tensor("qwT", [128, NCHUNK * L], bf16, kind="ExternalInput").ap()
    mtbd_d = nc.dram_tensor("mtbd", [P, 128], bf16, kind="ExternalInput").ap()
    mbd_d = nc.dram_tensor("mbd", [P, 128], bf16, kind="ExternalInput").ap()
    eyebd_d = nc.dram_tensor("eyebd", [P, 128], f32, kind="ExternalInput").ap()
    diagm2_d = nc.dram_tensor("diagm2", [P, RL], bf16, kind="ExternalInput").ap()
    ones_d = nc.dram_tensor("ones125", [P, 1], bf16, kind="ExternalInput").ap()
    outT = nc.dram_tensor("outT", [ROWS_PER_CORE * L, D], bf16,
                          kind="ExternalOutput").ap()

    with tile.TileContext(nc) as tc:
        with (
            tc.tile_pool(name="const", bufs=1) as const,
            tc.tile_pool(name="xpool", bufs=7) as xpool,
            tc.tile_pool(name="hpool", bufs=3) as hpool,
            tc.tile_pool(name="spool", bufs=3) as spool,
            tc.tile_pool(name="ps_ft", bufs=2, space=bass.MemorySpace.PSUM) as ps_ft,
            tc.tile_pool(name="ps_sc", bufs=2, space=bass.MemorySpace.PSUM) as ps_sc,
            tc.tile_pool(name="ps_bt", bufs=2, space=bass.MemorySpace.PSUM) as ps_bt,
            tc.tile_pool(name="ps_h", bufs=2, space=bass.MemorySpace.PSUM) as ps_h,
        ):
            mtbd = const.tile([P, 128], bf16)
            nc.sync.dma_start(mtbd[:], mtbd_d[:])
            mbd = const.tile([P, 128], bf16)
            nc.sync.dma_start(mbd[:], mbd_d[:])
            eyebd = const.tile([P, 128], f32)
            nc.sync.dma_start(eyebd[:], eyebd_d[:])
            diagm2 = const.tile([P, RL], bf16)
            nc.sync.dma_start(diagm2[:], diagm2_d[:])
            ones125 = const.tile([P, 1], bf16)
            nc.sync.dma_start(ones125[:], ones_d[:])
            epsb = const.tile([P, 1], f32)
            nc.vector.memset(epsb[:], EPS)
            junk = const.tile([P, P], f32)

            # two persistent vt tiles (ping-pong); qw block written once
            vt_a = const.tile([128, NCHUNK * CW], bf16)
            vt_b = const.tile([128, NCHUNK * CW], bf16)
            vts = [vt_a, vt_b]
            for v in vts:
                nc.sync.dma_start(
                    v.rearrange("p (c w) -> p c w", w=CW)[:, :, 128 : 128 + L],
                    qwT_d.rearrange("p (c w) -> p c w", w=L),
                )

            starts = _batch_starts()
            X_tiles = {}

            def load(j):
                if j >= len(starts):
                    return
                row0 = starts[j]
                X = xpool.tile([P, D], bf16, name=f"X{j}", tag="X")
                nc.gpsimd.dma_start(X[:], loT[row0 * NJ : row0 * NJ + P, :])
                X_tiles[j] = X

            def ft_part(i):
                X = X_tiles[i]
                vt3 = vts[i % 2].rearrange("p (c w) -> p c w", w=CW)
                for half in range(4):
                    ftp = ps_ft.tile([128, 512], f32, name=f"ftp{i}_{half}",
                                     tag="ft")
                    for cc in range(4):
                        c = 4 * half + cc
                        nc.tensor.matmul(
                            ftp[:, 128 * cc : 128 * (cc + 1)],
                            X[:, 128 * c : 128 * (c + 1)],
                            mtbd[:],
                            start=True,
                            stop=True,
                        )
                    ft4 = ftp.rearrange("p (cc w) -> p cc w", w=128)
                    dst = vt3[:, 4 * half : 4 * half + 4, 0:128]
                    if half % 2 == 0:
                        nc.scalar.copy(dst, ft4)
                    else:
                        nc.vector.tensor_copy(dst, ft4)

            def sc_part(i):
                vt3 = vts[i % 2].rearrange("p (c w) -> p c w", w=CW)
                SCp = ps_sc.tile([128, CW], f32, name=f"SCp{i}", tag="sc")
                for c in range(NCHUNK):
                    nc.tensor.matmul(
                        SCp[0:P, :],
                        vt3[:, c, 0:P],
                        vt3[:, c, 0:CW],
                        start=(c == 0),
                        stop=(c == NCHUNK - 1),
                    )
                return SCp

            def scores_part(i, SCp):
                ssq = spool.tile([P, 1], f32, name=f"ssq{i}", tag="ssq")
                nc.vector.scalar_tensor_tensor(
                    out=junk[:, 0:P],
                    in0=SCp[0:P, 0:P],
                    scalar=1.0,
                    in1=eyebd[:, 0:P],
                    op0=AluOpType.mult,
                    op1=AluOpType.mult,
                    accum_out=ssq[:],
                )
                lnu = spool.tile([P, 1], f32, name=f"lnu{i}", tag="lnu")
                nc.scalar.activation(
                    lnu[:], ssq[:], mybir.ActivationFunctionType.Ln,
                    bias=epsb[:], scale=1.0 / D,
                )
                rsq = spool.tile([P, 1], f32, name=f"rsq{i}", tag="rsq")
                nc.scalar.activation(
                    rsq[:], lnu[:], mybir.ActivationFunctionType.Exp, scale=-0.5
                )
                exp_s = spool.tile([P, L], bf16, name=f"exps{i}", tag="exps")
                nc.scalar.activation(
                    exp_s[:], SCp[0:P, 128 : 128 + L],
                    mybir.ActivationFunctionType.Exp, scale=rsq[:],
                )
                esc2 = spool.tile([P, RL], bf16, name=f"esc2{i}", tag="esc2")
                nc.gpsimd.tensor_tensor(
                    esc2.rearrange("p (r l) -> p r l", r=R),
                    exp_s.unsqueeze(1).broadcast_to([P, R, L]),
                    diagm2.rearrange("p (r l) -> p r l", r=R),
                    AluOpType.mult,
                )
                return esc2

            def bt_part(i, esc2):
                bt = ps_bt.tile([128, 128], f32, name=f"bt{i}", tag="bt")
                nc.tensor.matmul(bt[:, 0:RL], mbd[:], esc2[:],
                                 start=True, stop=True)
                nc.tensor.matmul(bt[0:RL, 120:121], esc2[:], ones125[:],
                                 start=True, stop=True)
                rec = spool.tile([RL, 1], f32, name=f"rec{i}", tag="rec")
                nc.vector.reciprocal(rec[:], bt[0:RL, 120:121])
                btsb = spool.tile([P, RL], bf16, name=f"btsb{i}", tag="btsb")
                nc.vector.tensor_copy(btsb[:], bt[0:P, 0:RL])
                return dict(rec=rec, btsb=btsb)

            def h_part(i, row0, bts):
                X, rec, btsb = X_tiles.pop(i), bts["rec"], bts["btsb"]
                h_sb = hpool.tile([RL, D], bf16, name=f"hsb{i}", tag="h")
                for nb in range(4):
                    Hp = ps_h.tile([128, 512], f32, name=f"Hp{i}_{nb}", tag="hp")
                    nc.tensor.matmul(
                        Hp[0:RL, :],
                        btsb[:],
                        X[:, 512 * nb : 512 * (nb + 1)],
                        start=True,
                        stop=True,
                    )
                    dst = h_sb[:, 512 * nb : 512 * (nb + 1)]
                    if nb < 2:
                        nc.scalar.activation(
                            dst, Hp[0:RL, :],
                            mybir.ActivationFunctionType.Copy, scale=rec[:],
                        )
                    else:
                        nc.vector.scalar_tensor_tensor(
                            out=dst,
                            in0=Hp[0:RL, :],
                            scalar=1.0,
                            in1=rec.broadcast_to([RL, 512]),
                            op0=AluOpType.mult,
                            op1=AluOpType.mult,
                        )
                nc.scalar.dma_start(
                    outT[row0 * L : row0 * L + RL, :], h_sb[:]
                )

            PF = 3   # X prefetch depth
            LAG = 2  # batches between scores and BT/H consumption
            for j in range(PF):
                load(j)
            pend = []  # [(i, esc2), ...] oldest first
            for i, row0 in enumerate(starts):
                ft_part(i)
                load(i + PF)
                if len(pend) >= LAG:
                    bts = bt_part(pend[0][0], pend[0][1])
                SCp = sc_part(i)
                if len(pend) >= LAG:
                    j0 = pend.pop(0)[0]
                    h_part(j0, starts[j0], bts)
                esc2 = scores_part(i, SCp)
                pend.append((i, esc2))
            while pend:
                bts = bt_part(pend[0][0], pend[0][1])
                j0 = pend.pop(0)[0]
                h_part(j0, starts[j0], bts)

    real_gat = bacc.get_activation_tables
    AF = mybir.ActivationFunctionType

    def gat_pinned(arch):
        out = {}
        for name, fns in real_gat(arch).items():
            if name == "natural_log_exp_and_others":
                out[name] = set(fns)
            else:
                out[name] = {f for f in fns if f not in (AF.Ln, AF.Exp)}
        return out

    bacc.get_activation_tables = gat_pinned
    try:
        if do_compile:
            nc.compile()
    finally:
        bacc.get_activation_tables = real_gat
    return nc


_NC_CACHE = None


def _prep_loT(layer_outputs, embedding):
    loT = np.empty((B * T, NJ, D), dtype=BF)
    loT[:, 0, :] = embedding.reshape(B * T, D).astype(BF)
    loT[:, 1:, :] = (
        layer_outputs.reshape(L, B * T, D).transpose(1, 0, 2).astype(BF)
    )
    return loT


def _make_in_maps(layer_outputs, embedding, queries, key_norm_weight):
    loT = _prep_loT(layer_outputs, embedding)
    consts = _build_consts(queries, key_norm_weight)
    in_maps = []
    for c in range(N_CORES):
        r0 = c * ROWS_PER_CORE
        in_maps.append({
            "loT": loT[r0 : r0 + ROWS_PER_CORE].reshape(ROWS_PER_CORE * NJ, D),
            "qwT": consts["qwT"],
            "mtbd": consts["mtbd"],
            "mbd": consts["mbd"],
            "eyebd": consts["eyebd"],
            "diagm2": consts["diagm2"],
            "ones125": consts["ones125"],
        })
    return in_maps


def kernel(layer_outputs, embedding, queries, key_norm_weight):
    global _NC_CACHE
    layer_outputs = np.asarray(layer_outputs, dtype=np.float32)
    embedding = np.asarray(embedding, dtype=np.float32)
    queries = np.asarray(queries, dtype=np.float32)
    key_norm_weight = np.asarray(key_norm_weight, dtype=np.float32)

    in_maps = _make_in_maps(layer_outputs, embedding, queries, key_norm_weight)

    if _NC_CACHE is None:
        _NC_CACHE = build_kernel()
    nc = _NC_CACHE

    res = run_bass_kernel_spmd(nc, in_maps, core_ids=list(range(N_CORES)))

    full = np.empty((L, B * T, D), dtype=np.float32)
    for c in range(N_CORES):
        r0 = c * ROWS_PER_CORE
        outT = res.results[c]["outT"].astype(np.float32).reshape(
            ROWS_PER_CORE, L, D
        )
        full[:, r0 : r0 + ROWS_PER_CORE, :] = outT.transpose(1, 0, 2)
    return full.reshape(L, B, T, D)
